# revision 26
# baseline (speedup 1.0000x reference)
"""DeepPoly SPU transformer — Trainium2 Bass kernel.

Elementwise over N=16777216; sharded across 8 NeuronCores (2M elems each,
viewed as [128 partitions x 16384 free]).

Math (per element; Z = sqrt(0.5)):
  spu(t)  = t^2 - 0.5 (t>=0) | -sigmoid(t) (t<0)      [== sigmoid(-t)-1]
  Cases:  A: u<=0   B: l>=0   C: l<0 & u>=Z   D: l<0 & 0<u<Z
  out       = spu(x) = relu(x)^2 - sigmoid(-relu(-x))
  new_upper = A: sl | B: su+1 | C,D: max(sl, su)   (chord value at u is su;
              flat4 in D picks max; A is always "flat" => sl)
        computed as: max(sigmoid(-l), u^2+0.5) -1 +[l>=0], CP A-> sigmoid(-l), -1 folded
  new_lower = A: sl | else: l^2-0.5-(G-l)^2 with G = B: a2 | C: max(a2,Z) | D: 0
        (tangent to t^2-0.5 at t=G; G=0 reproduces D's constant -0.5)
All identities verified against the jax reference to ~1e-7 * scale.
"""

import numpy as np

import concourse.bass as bass
import concourse.bacc as bacc
import concourse.mybir as mybir
from concourse.tile import TileContext
from concourse.bass_utils import run_bass_kernel_spmd

_N = 16777216
_NCORES = 8
_P = 128
_FDT = _N // _NCORES // _P  # 16384 free elems per partition per core
_FD = 2048                  # free-dim tile size
_NT = _FDT // _FD

_SQRT_HALF = float(np.float32(np.sqrt(0.5)))
_SQRT_TWO = float(np.float32(np.sqrt(2.0)))

_AF = mybir.ActivationFunctionType
_OP = mybir.AluOpType
_DT = mybir.dt.float32


def _build_nc(fd=_FD, io_bufs=3, tmp_bufs=2, fdt=_FDT, pool_masks=True,
              pe_ops=(), psum_bufs=2, aff="act", aff_out="pool", a2z2_pool=True, nl_direct=False, mz_dve=False, ramp=False, dma_prio=None):
    aff_out = aff if aff_out is None else aff_out
    pe_adds = bool(pe_ops)
    from contextlib import ExitStack

    nc = bacc.Bacc(trn_type="TRN2", debug=False, num_devices=_NCORES)
    nt = fdt // fd
    t_l = nc.dram_tensor("lb", [nt, _P, fd], _DT, kind="ExternalInput")
    t_u = nc.dram_tensor("ub", [nt, _P, fd], _DT, kind="ExternalInput")
    t_x = nc.dram_tensor("xx", [nt, _P, fd], _DT, kind="ExternalInput")
    t_o = nc.dram_tensor("o_spu", [nt, _P, fd], _DT, kind="ExternalOutput")
    t_nl = nc.dram_tensor("o_nl", [nt, _P, fd], _DT, kind="ExternalOutput")
    t_nu = nc.dram_tensor("o_nu", [nt, _P, fd], _DT, kind="ExternalOutput")

    if pe_adds:
        ident = np.eye(_P, dtype=np.float32)
        t_wI = nc.inline_tensor(ident, name="w_ident")
        t_wN = nc.inline_tensor(-ident, name="w_negident")
        t_wH = nc.inline_tensor(0.5 * ident, name="w_halfident")
    me = nc.gpsimd if pool_masks else nc.vector  # engine for masks + final affine
    with TileContext(nc) as tc, ExitStack() as ctx:
        iop = ctx.enter_context(tc.tile_pool(name="io", bufs=io_bufs))
        tp = ctx.enter_context(tc.tile_pool(name="tmp", bufs=tmp_bufs))
        if pe_adds:
            pp = ctx.enter_context(
                tc.tile_pool(name="ps", bufs=psum_bufs, space="PSUM"))
            cp = ctx.enter_context(tc.tile_pool(name="const", bufs=1))
            wI = cp.tile([_P, _P], _DT, tag="wI")
            nc.sync.dma_start(out=wI[:], in_=t_wI[:, :])
            wN = cp.tile([_P, _P], _DT, tag="wN")
            nc.sync.dma_start(out=wN[:], in_=t_wN[:, :])
            wH = cp.tile([_P, _P], _DT, tag="wH")
            nc.sync.dma_start(out=wH[:], in_=t_wH[:, :])

        def pe_acc2(pt, w0, r0, w1, r1):
            # pt = w0.T @ r0 + w1.T @ r1 in 512-wide slices (1 PSUM bank each).
            # Weights-outer order: one LDWEIGHTS per weight instead of per slice.
            for w, r, st in ((w0, r0, True), (w1, r1, False)):
                for j in range(0, fd, 512):
                    sl = (slice(None), slice(j, j + 512))
                    nc.tensor.matmul(pt[sl], w[:], r[sl],
                                     start=st, stop=not st)

        if ramp == "start":
            chunks = [(0, c, fd // 2) for c in range(0, fd, fd // 2)]
            chunks += [(i, 0, fd) for i in range(1, nt)]
        elif ramp:
            chunks = [(0, c, fd // 4) for c in range(0, fd, fd // 4)]
            chunks += [(i, 0, fd) for i in range(1, nt - 1)]
            chunks += [(nt - 1, c, fd // 2) for c in range(0, fd, fd // 2)]
        else:
            chunks = [(i, 0, fd) for i in range(nt)]
        for (i, c0, fdc) in chunks:
            cols = (i, slice(None), slice(c0, c0 + fdc))

            from contextlib import nullcontext
            with (tc.high_priority(dma_prio) if dma_prio is not None else nullcontext()):
                l = iop.tile([_P, fdc], _DT, tag="l")
                nc.sync.dma_start(out=l[:], in_=t_l[cols])
                u = iop.tile([_P, fdc], _DT, tag="u")
                nc.sync.dma_start(out=u[:], in_=t_u[cols])
                x = iop.tile([_P, fdc], _DT, tag="x")
                nc.sync.dma_start(out=x[:], in_=t_x[cols])

            # --- ACT chain ---
            s2l = tp.tile([_P, fdc], _DT, tag="s2l")
            nc.scalar.activation(s2l[:], l[:], _AF.Sigmoid, scale=-1.0)  # sigmoid(-l)
            usq = tp.tile([_P, fdc], _DT, tag="usq")
            nc.scalar.activation(usq[:], u[:], _AF.Relu)                 # relu(u)
            nc.scalar.activation(usq[:], usq[:], _AF.Square)             # relu(u)^2
            lsq = tp.tile([_P, fdc], _DT, tag="lsq")
            nc.scalar.activation(lsq[:], l[:], _AF.Square)               # l^2
            sx = tp.tile([_P, fdc], _DT, tag="sx")
            nc.scalar.activation(sx[:], x[:], _AF.Sigmoid, scale=-1.0)   # sigmoid(-x)
            rx = tp.tile([_P, fdc], _DT, tag="rx")
            nc.scalar.activation(rx[:], x[:], _AF.Relu)                  # relu(x)
            nc.scalar.activation(rx[:], rx[:], _AF.Square)               # relu(x)^2

            # --- masks (1 / 0, uint8: CopyPredicated needs int dtype) ---
            mA = tp.tile([_P, fdc], mybir.dt.uint8, tag="mA")
            me.tensor_scalar(mA[:], u[:], 0.0, None, _OP.is_le)
            mB = tp.tile([_P, fdc], mybir.dt.uint8, tag="mB")
            me.tensor_scalar(mB[:], l[:], 0.0, None, _OP.is_ge)
            mZ = tp.tile([_P, fdc], mybir.dt.uint8, tag="mZ")
            (nc.vector if mz_dve else me).tensor_scalar(
                mZ[:], u[:], _SQRT_HALF, None, _OP.is_ge)

            # --- s2 = u + l ---
            if "s2" in pe_ops:
                s2 = pp.tile([_P, fdc], _DT, tag="ps")
                pe_acc2(s2, wI, u, wI, l)
            else:
                s2 = tp.tile([_P, fdc], _DT, tag="s2")
                nc.vector.tensor_tensor(s2[:], u[:], l[:], _OP.add)

            # --- G chain (g holds 2*G, then (G-l)^2) ---
            g = tp.tile([_P, fdc], _DT, tag="g")
            (nc.gpsimd if a2z2_pool else nc.vector).tensor_scalar(
                g[:], s2[:], _SQRT_TWO, None, _OP.max)                   # max(u+l, 2Z)
            me.tensor_tensor(g[:], g[:], mZ[:], _OP.mult)                # 0 unless u>=Z
            nc.vector.copy_predicated(g[:], mB[:], s2[:])                # B rows: u+l
            if "gl" in pe_ops:
                gl = pp.tile([_P, fdc], _DT, tag="ps")
                pe_acc2(gl, wH, g, wN, l)                                # G - l
                nc.scalar.activation(g[:], gl[:], _AF.Square)            # (G-l)^2
            else:
                nc.vector.scalar_tensor_tensor(
                    g[:], g[:], 0.5, l[:], _OP.mult, _OP.subtract)       # G - l
                nc.scalar.activation(g[:], g[:], _AF.Square)             # (G-l)^2

            if nl_direct:
                # direct space: nl = (l^2 - 0.5) - (G-l)^2; A-override with
                # sl = sigmoid(-l) - 1 materialized off-chain on GPSIMD
                slt = tp.tile([_P, fdc], _DT, tag="slt")
                nc.gpsimd.tensor_scalar(slt[:], s2l[:], 1.0, None, _OP.subtract)
                nc.vector.scalar_tensor_tensor(
                    lsq[:], lsq[:], -0.5, g[:], _OP.add, _OP.subtract)   # l^2-0.5-(G-l)^2
                nc.vector.copy_predicated(lsq[:], mA[:], slt[:])         # A: sl
            else:
                nc.vector.scalar_tensor_tensor(
                    lsq[:], lsq[:], 0.5, g[:], _OP.add, _OP.subtract)    # l^2+0.5-(G-l)^2
                nc.vector.copy_predicated(lsq[:], mA[:], s2l[:])         # A: sigmoid(-l)
                if aff == "pool":
                    nc.gpsimd.tensor_scalar(lsq[:], lsq[:], 1.0, None, _OP.subtract)
                elif aff == "dve":
                    nc.vector.tensor_scalar(lsq[:], lsq[:], 1.0, None, _OP.subtract)
                else:
                    nc.scalar.activation(lsq[:], lsq[:], _AF.Copy, bias=-1.0)

            # --- new_upper (in usq; +1 space) ---
            # max(relu(u)^2+0.5, sigmoid(-l)): A rows (u<=0) give relu(u)=0 ->
            # 0.5 <= sigmoid(-l), so the max already selects sl there.
            nc.vector.scalar_tensor_tensor(
                usq[:], usq[:], 0.5, s2l[:], _OP.add, _OP.max)
            nc.vector.scalar_tensor_tensor(
                usq[:], usq[:], -1.0, mB[:], _OP.add, _OP.add)           # -1 + [l>=0]

            # --- out: out+1 = max(sigmoid(-x), relu(x)^2 + 0.5) ---
            o = rx
            nc.vector.scalar_tensor_tensor(
                rx[:], rx[:], 0.5, sx[:], _OP.add, _OP.max)
            if aff_out == "pool":
                nc.gpsimd.tensor_scalar(o[:], o[:], 1.0, None, _OP.subtract)
            elif aff_out == "dve":
                nc.vector.tensor_scalar(o[:], o[:], 1.0, None, _OP.subtract)
            else:
                nc.scalar.activation(o[:], o[:], _AF.Copy, bias=-1.0)

            nc.sync.dma_start(out=t_o[cols], in_=o[:])
            nc.sync.dma_start(out=t_nl[cols], in_=lsq[:])
            nc.sync.dma_start(out=t_nu[cols], in_=usq[:])
    nc.compile()
    return nc


_NC_CACHE = {}


def _get_nc(**kw):
    key = tuple(sorted(kw.items()))
    if key not in _NC_CACHE:
        _NC_CACHE[key] = _build_nc(**kw)
    return _NC_CACHE[key]


def _run(x, lower_bounds, upper_bounds, trace=False, **build_kw):
    assert x.shape == (_N,) and x.dtype == np.float32
    nc = _get_nc(**build_kw)
    fd = build_kw.get("fd", _FD)
    nt = _FDT // fd
    shp = (_NCORES, nt, _P, fd)
    ls = np.ascontiguousarray(lower_bounds.reshape(shp))
    us = np.ascontiguousarray(upper_bounds.reshape(shp))
    xs = np.ascontiguousarray(x.reshape(shp))
    in_maps = [{"lb": ls[c], "ub": us[c], "xx": xs[c]} for c in range(_NCORES)]
    res = run_bass_kernel_spmd(
        nc, in_maps, core_ids=list(range(_NCORES)), trace=trace
    )
    out = np.concatenate([res.results[c]["o_spu"].reshape(-1) for c in range(_NCORES)])
    nl = np.concatenate([res.results[c]["o_nl"].reshape(-1) for c in range(_NCORES)])
    nu = np.concatenate([res.results[c]["o_nu"].reshape(-1) for c in range(_NCORES)])
    return (out, nl, nu), res


def kernel(x, lower_bounds, upper_bounds):
    (out, nl, nu), _ = _run(x, lower_bounds, upper_bounds)
    return (out, nl, nu)
